# revision 50
# baseline (speedup 1.0000x reference)
"""Paged-KV-cache GQA attention with int8 tensor-cast quantization, TRN2.

Sharding: tensor-parallel over KV heads. Core c owns kv-head c and its G=4
query heads. Host does index-driven data movement (scatter new quantized
K/V into the paged cache, gather pages via block_table) plus exact int8
quantization; the device does all attention compute.

Device pipeline per (b, g, lqc) iteration (32 per core):
  scores[q, s] = Q @ K^T   (PE f16 exact-int matmuls -> PSUM f32; the causal
                            mask is an extra PE matmul, lower-triangular ones
                            x shifted-delta = -1e9 * [j > q], accumulated)
  e = exp(C * scores)      (ACT, 3 big PSUM reads, accum_out row sums)
  p127 = RNE(e * r127)     (one DVE op: float->int16 conversion rounds to
                            nearest-even at fp32 precision, matching
                            np.round; then an exact int16 -> f16 4x copy)
  pT = transpose(p)        (multi-tile DMA transposes, 2 per iteration)
  out[q, d] = (pT @ V) * ps*ks   (PE accumulate, DVE evict, f16 out)

K/V ship as int8 and are cast to f16 on the Pool engine. All matmul
operands are integers held exactly in f16, so PSUM dot products are exact.
PSUM layout (16KB/partition = 4096 f32): scores A[2048] B[1536] C[512],
with the C bank time-shared against the PV accumulator via a bufs=1
same-tag pool (C-chunk QK+exp run first in each block so the PV batch
slots in off the critical path). QK runs one step ahead of the softmax
stage; the PV+output stage trails LAG iterations behind."""

import sys

sys.path.insert(0, "/opt/trn_rl_repo")

import numpy as np

import concourse.bass as bass
import concourse.mybir as mybir
from concourse import tile
from concourse.bass_utils import run_bass_kernel_spmd

# Problem dims (hardcoded per spec)
B, H, KVH, D = 4, 32, 8, 128
LQ, S, BLOCK = 256, 4096, 16
BPS = S // BLOCK
NUM_BLOCKS = B * BPS
SLOTS = NUM_BLOCKS * BLOCK
T = B * LQ
G = H // KVH
N_CORES = 8
NCH = S // 128
SM_SCALE = 1.0 / float(np.sqrt(D))
LAG = 2  # software-pipeline distance of the PV stage

F16 = np.float16

_CACHE = {}


def _build(C, inv_ps, out_scale, k_off):
    nc = bass.Bass()
    f16 = mybir.dt.float16
    f32 = mybir.dt.float32
    i8 = mybir.dt.int8
    i16 = mybir.dt.int16
    i32 = mybir.dt.int32
    AF = mybir.ActivationFunctionType
    OP = mybir.AluOpType
    X = mybir.AxisListType.X

    blob8 = nc.dram_tensor("blob8", [B, 128, S + NCH * D], i8, kind="ExternalInput")
    blobh = nc.dram_tensor("blobh", [B, 128, G * LQ], f16, kind="ExternalInput")
    out = nc.dram_tensor("out", [B, 128, G * 2 * D], f16, kind="ExternalOutput")

    with tile.TileContext(nc) as tc:
        with (
            tc.tile_pool(name="const", bufs=1) as constp,
            tc.tile_pool(name="b8", bufs=2) as b8p,
            tc.tile_pool(name="kh", bufs=2) as khp,
            tc.tile_pool(name="vh", bufs=2) as vhp,
            tc.tile_pool(name="qh", bufs=2) as qhp,
            tc.tile_pool(name="ef", bufs=2) as efp,
            tc.tile_pool(name="pi", bufs=2) as pip,
            tc.tile_pool(name="pti", bufs=2) as ptip,
            tc.tile_pool(name="ptf", bufs=LAG + 2) as ptfp,
            tc.tile_pool(name="small", bufs=8) as smallp,
            tc.tile_pool(name="outs", bufs=2) as outsp,
            tc.tile_pool(name="psA", bufs=1, space="PSUM") as psAp,
            tc.tile_pool(name="psB", bufs=1, space="PSUM") as psBp,
            tc.tile_pool(name="b7", bufs=1, space="PSUM") as b7p,
        ):
            # causal mask via an extra PE matmul accumulated into PSUM:
            # Utri[k,p] = [p <= k];  Wl[k,j] = -1e9 * [j - k == off]
            # (U.T @ W)[p,j] = -1e9 * [p <= j - off]  == causal mask window
            bf = mybir.dt.bfloat16
            rampU = constp.tile([128, 128], i32)
            nc.gpsimd.iota(rampU[:], pattern=[[-1, 128]], base=0, channel_multiplier=1)
            tU = constp.tile([128, 128], f32)
            nc.vector.tensor_scalar(tU[:], rampU[:], 0.0, None, OP.is_ge)
            utri = constp.tile([128, 128], bf)
            nc.vector.tensor_copy(utri[:], tU[:])
            rampW = constp.tile([128, LQ], i32)
            nc.gpsimd.iota(rampW[:], pattern=[[1, LQ]], base=0, channel_multiplier=-1)
            wmask = []
            for lqc in range(2):
                off = 129 if lqc == 1 else 1
                t = constp.tile([128, LQ], f32, tag=f"wt{lqc}")
                nc.vector.tensor_scalar(
                    t[:], rampW[:], float(off), 0.0, OP.subtract, OP.is_equal
                )
                w = constp.tile([128, LQ], bf, tag=f"wm{lqc}")
                nc.vector.tensor_scalar(w[:], t[:], -1.0e9, None, OP.mult)
                wmask.append(w)

            tiles = {}

            def load_blob(b):
                k8 = b8p.tile([128, S], i8, tag="k8")
                kh = khp.tile([128, S], f16)
                qh = qhp.tile([128, G * LQ], f16)
                if b == 0:
                    # first blob is on the critical path: C-region K piece and
                    # the first query chunk land first, staged finely
                    nc.sync.dma_start(out=k8[:, 3584:4096], in_=blob8[b, :, 3584:4096])
                    nc.gpsimd.tensor_scalar(
                        kh[:, 3584:4096], k8[:, 3584:4096], float(k_off), None, OP.subtract
                    )
                    nc.sync.dma_start(out=qh[:, 0:128], in_=blobh[b, :, 0:128])
                    nc.sync.dma_start(out=qh[:, 128:], in_=blobh[b, :, 128:])
                    pieces = ((2048, 3584), (0, 2048))
                else:
                    nc.sync.dma_start(out=qh[:], in_=blobh[b])
                    pieces = ((2048, 4096), (0, 2048))
                for c0, c1 in pieces:
                    nc.sync.dma_start(out=k8[:, c0:c1], in_=blob8[b, :, c0:c1])
                    nc.gpsimd.tensor_scalar(
                        kh[:, c0:c1], k8[:, c0:c1], float(k_off), None, OP.subtract
                    )
                v8 = b8p.tile([128, NCH * D], i8, tag="v8")
                nc.sync.dma_start(out=v8[:], in_=blob8[b, :, S : S + NCH * D])
                vh = vhp.tile([128, NCH * D], f16)
                nc.gpsimd.tensor_scalar(vh[:], v8[:], float(k_off), None, OP.subtract)
                tiles[b] = (kh, vh, qh)

            pend_qk = {}
            pend_soft = {}

            def dec(it):
                lqc = it % 2 if it < B * G * 2 - 2 else 1 - (it % 2)
                return it // 8, (it % 8) // 2, lqc

            def emit_qk(it):
                b, g, lqc = dec(it)
                if it == 0:
                    load_blob(0)
                if it % 8 == 4 and b + 1 < B:
                    load_blob(b + 1)
                kh, vh, qh = tiles[b]
                ncols = 4096 if lqc == 1 else 3968
                qa = g * LQ + lqc * 128

                scC = b7p.tile([128, 512], f32, tag="cbank")
                scA = psAp.tile([128, 2048], f32)
                scB = psBp.tile([128, 1536], f32)
                nc.tensor.matmul(
                    scC[:, 0:256],
                    qh[:, qa : qa + 128],
                    kh[:, 3584:3840],
                    start=True,
                    stop=True,
                )
                nc.tensor.matmul(
                    scC[:, 256 : ncols - 3584],
                    qh[:, qa : qa + 128],
                    kh[:, 3840:ncols],
                    start=True,
                    stop=False,
                )
                # causal mask accumulated on PE: adds -1e9 where j > q
                nc.tensor.matmul(
                    scC[:, 256 : ncols - 3584],
                    utri[:],
                    wmask[lqc][:, 0 : ncols - 3840],
                    start=False,
                    stop=True,
                )
                for k in range(4):
                    nc.tensor.matmul(
                        scA[:, 512 * k : 512 * (k + 1)],
                        qh[:, qa : qa + 128],
                        kh[:, 512 * k : 512 * (k + 1)],
                        start=True,
                        stop=True,
                    )
                for k in range(3):
                    nc.tensor.matmul(
                        scB[:, 512 * k : 512 * (k + 1)],
                        qh[:, qa : qa + 128],
                        kh[:, 2048 + 512 * k : 2048 + 512 * (k + 1)],
                        start=True,
                        stop=True,
                    )
                pend_qk[it] = (scA, scB, scC)

            def emit_soft(it):
                b, g, lqc = dec(it)
                ncols = 4096 if lqc == 1 else 3968
                scA, scB, scC = pend_qk.pop(it)

                ef = efp.tile([128, S], f32)
                acc = smallp.tile([128, 4], f32, tag="acc")
                nc.scalar.activation(
                    ef[:, 3584:ncols],
                    scC[:, 0 : ncols - 3584],
                    AF.Exp,
                    scale=C,
                    accum_out=acc[:, 2:3],
                )
                nc.scalar.activation(
                    ef[:, 0:2048], scA[:], AF.Exp, scale=C, accum_out=acc[:, 0:1]
                )
                nc.scalar.activation(
                    ef[:, 2048:3584], scB[:], AF.Exp, scale=C, accum_out=acc[:, 1:2]
                )
                sumv = smallp.tile([128, 1], f32, tag="sumv")
                nc.vector.tensor_reduce(sumv[:], acc[:, 0:3], X, OP.add)
                rv = smallp.tile([128, 1], f32, tag="rv")
                nc.vector.reciprocal(rv[:], sumv[:])
                r127 = smallp.tile([128, 1], f32, tag="r127")
                nc.vector.tensor_scalar_mul(r127[:], rv[:], inv_ps)

                # p127 = RNE(e * r127): DVE float->int16 conversion rounds
                # to nearest-even at fp32 internal precision (verified on
                # HW), then the int16 -> f16 copy is exact for ints <= 127.
                pi = pip.tile([128, S], i16)
                pti = ptip.tile([128, S], i16)
                ptf = ptfp.tile([128, S], f16)
                if it == B * G * 2 - 1:
                    bounds = ((0, 1024), (1024, 2048), (2048, 3072), (3072, 3840), (3840, ncols))
                else:
                    bounds = ((0, 1024), (1024, 2048), (2048, 3072), (3072, ncols))
                for h, (c0, c1) in enumerate(bounds):
                    nc.vector.tensor_scalar(
                        pi[:, c0:c1], ef[:, c0:c1], r127[:], None, OP.mult
                    )
                    nc.sync.dma_start_transpose(
                        out=pti[:, c0:c1].rearrange("p (c q) -> p c q", c=(c1 - c0) // 128),
                        in_=pi[:, c0:c1],
                    )
                    nc.vector.tensor_copy(ptf[:, c0:c1], pti[:, c0:c1])
                pend_soft[it] = ptf

            def emit_pv(j):
                b, g, lqc = dec(j)
                ncols = 4096 if lqc == 1 else 3968
                nch = ncols // 128
                kh, vh, qh = tiles[b]
                pt = pend_soft.pop(j)  # transposed f16 p-matrix
                pvt = b7p.tile([128, 512], f32, tag="cbank")
                pv = pvt[:, 0:128]
                for c in range(nch):
                    nc.tensor.matmul(
                        pv,
                        pt[:, c * 128 : (c + 1) * 128],
                        vh[:, c * D : (c + 1) * D],
                        start=(c == 0),
                        stop=(c == nch - 1),
                    )
                if j % 8 == 0:
                    ob = outsp.tile([128, G * 2 * D], f16, tag="ob")
                    tiles[f"o{b}"] = ob
                ob = tiles[f"o{b}"]
                col = (g * 2 + lqc) * D
                nc.vector.tensor_scalar(
                    ob[:, col : col + D], pv, out_scale, None, OP.mult
                )
                if j == B * G * 2 - 1:
                    nc.sync.dma_start(out=out[b, :, 0 : col], in_=ob[:, 0:col])
                    nc.sync.dma_start(out=out[b, :, col:], in_=ob[:, col:])
                elif j % 8 == 7:
                    nc.sync.dma_start(out=out[b], in_=ob[:])

            NIT = B * G * 2
            for step in range(NIT + 1 + LAG):
                if step >= 1 + LAG:
                    emit_pv(step - 1 - LAG)
                if step < NIT:
                    emit_qk(step)
                if 1 <= step <= NIT:
                    emit_soft(step - 1)

    _legalize_waits(nc)
    return nc


def _legalize_waits(nc, maxw=1):
    """Walrus rejects instructions with too many sync waits. Move excess
    waits onto injected same-engine NoOps placed just before the
    instruction (engine program order preserved, so semantics identical)."""
    fixid = 0
    for bb in nc.main_func.blocks:
        insts = list(bb.instructions)
        changed = False
        newlist = []
        for ins in insts:
            si = ins.sync_info
            waits = list(si.on_wait) if si and si.on_wait else []
            if len(waits) > maxw:
                keep = waits[-maxw:]
                excess = waits[:-maxw]
                for j in range(0, len(excess), maxw):
                    nop = mybir.InstNoOp(name=f"I-waitfix-{fixid}", ins=[], outs=[])
                    fixid += 1
                    nop.engine = ins.engine
                    nop.sync_info = mybir.SyncInfo(
                        on_wait=excess[j : j + maxw], on_update=[]
                    )
                    newlist.append(nop)
                ins.sync_info = mybir.SyncInfo(
                    on_wait=keep,
                    on_update=list(si.on_update) if si.on_update else [],
                )
                changed = True
            newlist.append(ins)
        if changed:
            try:
                bb.instructions = newlist
            except Exception:
                bb.instructions.clear()
                bb.instructions.extend(newlist)
    return nc


def kernel(
    query,
    key,
    value,
    kv_cache,
    block_table,
    slot_mapping,
    query_start_loc,
    seq_lens,
    query_lens,
    q_scale,
    q_offset,
    kv_scale,
    kv_offset,
    prob_scale,
    prob_offset,
):
    query = np.asarray(query, np.float32)
    key = np.asarray(key, np.float32)
    value = np.asarray(value, np.float32)
    kv_cache = np.asarray(kv_cache)
    block_table = np.asarray(block_table, np.int32)
    slot_mapping = np.asarray(slot_mapping, np.int32)
    seq_lens = np.asarray(seq_lens, np.int32)
    qs = float(np.asarray(q_scale).reshape(-1)[0])
    qo = float(np.asarray(q_offset).reshape(-1)[0])
    ks = float(np.asarray(kv_scale).reshape(-1)[0])
    ko = float(np.asarray(kv_offset).reshape(-1)[0])
    ps = float(np.asarray(prob_scale).reshape(-1)[0])

    # quantize new K/V (same fp32 arithmetic as the reference)
    def quant(x, sc, off):
        return np.clip(
            np.round(x / np.float32(sc) + np.float32(off)), -128.0, 127.0
        ).astype(np.int8)

    k_q = quant(key, ks, ko)
    v_q = quant(value, ks, ko)
    flat = kv_cache.reshape(2, SLOTS, KVH, D).copy()
    flat[0, slot_mapping] = k_q
    flat[1, slot_mapping] = v_q
    cache = flat.reshape(2, NUM_BLOCKS, BLOCK, KVH, D)
    k_i8 = cache[0][block_table].reshape(B, S, KVH, D)  # int8
    v_i8 = cache[1][block_table].reshape(B, S, KVH, D)

    q_int = np.clip(np.round(query / np.float32(qs) + np.float32(qo)), -128.0, 127.0)
    q_eff = (q_int.astype(np.float32) - np.float32(qo)).astype(F16)
    q5 = q_eff.reshape(B, LQ, KVH, G, D)

    # the device applies the causal mask for the last LQ key columns only;
    # verify the requested mask matches that structure
    q_pos = seq_lens[:, None] - LQ + np.arange(LQ, dtype=np.int32)[None, :]
    k_pos = np.arange(S, dtype=np.int32)
    mask = (k_pos[None, None, :] <= q_pos[:, :, None]) & (
        k_pos[None, None, :] < seq_lens[:, None, None]
    )  # [B, LQ, S]
    assert mask[:, :, : S - LQ].all(), "dense prefix expected"
    jj = np.arange(LQ)[None, :]
    qq = np.arange(LQ)[:, None]
    want = jj <= qq
    assert (mask[:, :, S - LQ :] == want[None]).all(), "causal tail expected"

    C = float(qs * ks * SM_SCALE)
    inv_ps = float(1.0 / ps)
    out_scale = float(ps * ks)

    key_sig = (C, inv_ps, out_scale, ko)
    if key_sig not in _CACHE:
        _CACHE[key_sig] = _build(C, inv_ps, out_scale, ko)
    nc = _CACHE[key_sig]

    in_maps = []
    for c in range(N_CORES):
        b8 = np.empty((B, 128, S + NCH * D), np.int8)
        # KT int8: [D, S] per b
        b8[:, :, 0:S] = np.transpose(k_i8[:, :, c, :], (0, 2, 1))
        # V int8: [s-in-chunk, NCH*D] per b
        b8[:, :, S:] = (
            v_i8[:, :, c, :]
            .reshape(B, NCH, 128, D)
            .transpose(0, 2, 1, 3)
            .reshape(B, 128, NCH * D)
        )
        bh = np.ascontiguousarray(
            q5[:, :, c, :, :].transpose(0, 3, 2, 1).reshape(B, 128, G * LQ)
        )
        in_maps.append({"blob8": b8, "blobh": bh})

    global _LAST_IN_MAPS
    _LAST_IN_MAPS = in_maps
    res = run_bass_kernel_spmd(nc, in_maps, list(range(N_CORES)))
    # out[core][b, p, (g*2+lqc)*128 + d] -> full[b*LQ + lqc*128 + p, c*G+g, d]
    outs = np.stack([r["out"] for r in res.results])  # [KVH, B, 128, G*2*D]
    o = outs.reshape(KVH, B, 128, G, 2, D).astype(np.float32)
    full = np.transpose(o, (1, 4, 2, 0, 3, 5)).reshape(T, H, D)
    return np.ascontiguousarray(full)
